# revision 1
# baseline (speedup 1.0000x reference)
"""LocalAttention Trainium2 Bass kernel.

Problem: x (2, 2048, 1024) f32 -> QKV proj (16 heads, d=64), local-window
attention (|i-j| <= 128), output projection.

Sharding (no collectives): 8 shards = 2 batches x 4 seq-chunks of 512 q rows.
Each core gets a uniform 768-row KV halo slice [qs-128, qs+640), zero-padded
at the sequence edges, so the q rows sit at fixed offset 128 inside the kv
slice on every core (SPMD-uniform band positions; the mask also kills the
padded rows). Attention runs banded: per head and per 256-wide q slice, only
the 4 kv tiles (512 rows) covering the window are computed. Host
pre-transposes the x slices (x^T layout is required for PE contraction over
the embedding dim); output shards are disjoint row slices concatenated on
the host.

Device layouts per core:
  xtq [1024, 512]  x^T for q rows      xtkv [1024, 768]  x^T for kv rows
  QT  [1024, 512]  (head*64+d) x q     KT   [1024, 768]
  V~  [768, 16*65] per head 64 v-cols + ones col (softmax-sum trick)
  SET = exp(energy/32) * mask in [kv, q] layout -> PV matmul contracts kv on
  partitions; row 64 of the PV psum accumulates the softmax denominator.
  OT [1024, 512] normalized head outputs -> final projection vs wo.

All matmuls run as float32r (full-rate PE mode, fp32 storage); toggle with
KERNEL_F32R=0 for exact fp32 (4x slower PE).
"""

import os
from contextlib import ExitStack

import numpy as np

import concourse.bacc as bacc
import concourse.mybir as mybir
import concourse.tile as tile
from concourse.bass_utils import run_bass_kernel_spmd

F32 = mybir.dt.float32
F32R = mybir.dt.float32r
AF = mybir.ActivationFunctionType

EMB = 1024
NHEAD = 16
DHEAD = 64
WIN = 128
BATCH = 2
SEQ = 2048
NQ = 512          # q rows per core
NKV = 768         # kv rows per core (q rows + clamped 128 halo each side)
NT_E = EMB // 128   # 8 e-tiles
NT_KV = NKV // 128  # 6 kv-tiles
NT_Q = NQ // 128    # 4 q-tiles
QOFF = 128          # q row i == kv row i + QOFF, uniformly
NSL = 2             # q slices per core (256 wide)
QSL = NQ // NSL     # 256
SCALE = 1.0 / np.sqrt(EMB)       # energy / sqrt(emb)

USE_F32R = os.environ.get("KERNEL_F32R", "1") == "1"

LAST_RESULT = None  # BassKernelResults of the most recent run (for profiling)


def _body(ctx, tc, aps):
    nc = tc.nc
    MMDT = F32R if USE_F32R else F32
    r = lambda ap: ap  # matmul operands are typed F32R at declaration  # noqa: E731

    pw = ctx.enter_context(tc.tile_pool(name="w", bufs=9))
    pxt = ctx.enter_context(tc.tile_pool(name="xtot", bufs=8))
    pqt = ctx.enter_context(tc.tile_pool(name="qt", bufs=8))
    pkt = ctx.enter_context(tc.tile_pool(name="kt", bufs=8))
    pv = ctx.enter_context(tc.tile_pool(name="v", bufs=6))
    pset = ctx.enter_context(tc.tile_pool(name="set", bufs=14))
    pmask = ctx.enter_context(tc.tile_pool(name="mask", bufs=6))
    psml = ctx.enter_context(tc.tile_pool(name="small", bufs=1))
    prcb = ctx.enter_context(tc.tile_pool(name="rcb", bufs=3))
    pfin = ctx.enter_context(tc.tile_pool(name="fin", bufs=2))
    pbo = ctx.enter_context(tc.tile_pool(name="bo", bufs=1))
    pps = ctx.enter_context(tc.tile_pool(name="ps", bufs=5, space="PSUM"))
    ppv = ctx.enter_context(tc.tile_pool(name="pspv", bufs=3, space="PSUM"))

    ts = lambda i, s: slice(i * s, (i + 1) * s)  # noqa: E731

    # ---- persistent loads ----
    xtkv = []
    for t in range(NT_E):
        tk = pxt.tile([128, NKV], MMDT, tag="xtkv", bufs=8)
        nc.sync.dma_start(tk[:], aps["xtkv"][ts(t, 128), :])
        xtkv.append(tk)
    maskt = []
    for t in range(NT_KV):
        m = pmask.tile([128, NQ], F32, tag="mask")
        nc.sync.dma_start(m[:], aps["mask"][ts(t, 128), :])
        maskt.append(m)
    bqs = psml.tile([128, NT_E], F32, tag="bias", bufs=3)
    nc.sync.dma_start(bqs[:], aps["bqc"][:])
    bks = psml.tile([128, NT_E], F32, tag="bias", bufs=3)
    nc.sync.dma_start(bks[:], aps["bkc"][:])
    bvs = psml.tile([128, NT_E], F32, tag="bias", bufs=3)
    nc.sync.dma_start(bvs[:], aps["bvc"][:])
    bob = pbo.tile([128, EMB], F32, tag="bo")
    nc.sync.dma_start(bob[:], aps["bob"][:])

    def load_w(name):
        tiles = []
        for t in range(NT_E):
            w = pw.tile([128, EMB], MMDT, tag="w")
            nc.sync.dma_start(w[:], aps[name][ts(t, 128), :])
            tiles.append(w)
        return tiles

    # ---- QT = (x_q @ wq + bq)^T ----
    wq_t = load_w("wq")
    qt = []
    for t in range(NT_E):
        ps = pps.tile([128, NQ], F32, tag="ps")
        for k in range(NT_E):
            nc.tensor.matmul(ps[:], r(wq_t[k][:, ts(t, 128)]),
                 r(xtkv[k][:, QOFF:QOFF + NQ]),
                             start=(k == 0), stop=(k == NT_E - 1))
        q = pqt.tile([128, NQ], MMDT, tag="qt")
        nc.scalar.activation(q[:], ps[:], AF.Identity, bias=bqs[:, t:t + 1])
        qt.append(q)

    # ---- KT = (x_kv @ wk + bk)^T ----
    wk_t = load_w("wk")
    kt = []
    for t in range(NT_E):
        k_tile = pkt.tile([128, NKV], MMDT, tag="kt")
        for half in range(2):
            ps = pps.tile([128, NKV // 2], F32, tag="ps")
            for k in range(NT_E):
                nc.tensor.matmul(ps[:], r(wk_t[k][:, ts(t, 128)]),
                                 r(xtkv[k][:, ts(half, NKV // 2)]),
                                 start=(k == 0), stop=(k == NT_E - 1))
            nc.scalar.activation(k_tile[:, ts(half, NKV // 2)], ps[:],
                                 AF.Identity, bias=bks[:, t:t + 1])
        kt.append(k_tile)

    # ---- V~ = x_kv @ wv, strided per head with a ones column ----
    wv_t = load_w("wv")
    v = []
    for kvt in range(NT_KV):
        vt = pv.tile([128, NHEAD * (DHEAD + 1)], MMDT, tag="v")
        vt_r = vt[:].rearrange("p (h d) -> p h d", d=DHEAD + 1)
        nc.sync.dma_start(vt_r[:, :, DHEAD:DHEAD + 1],
                          aps["onesc"][:].unsqueeze(2))
        for half in range(2):
            ps = pps.tile([128, 512], F32, tag="ps")
            for k in range(NT_E):
                nc.tensor.matmul(ps[:], r(xtkv[k][:, ts(kvt, 128)]),
                                 r(wv_t[k][:, ts(half, 512)]),
                                 start=(k == 0), stop=(k == NT_E - 1))
            nc.vector.tensor_copy(
                vt_r[:, ts(half, 8), 0:DHEAD],
                ps[:].rearrange("p (h d) -> p h d", d=DHEAD))
        v.append(vt)

    # ---- attention per head ----
    wo_t = load_w("wo")  # prefetch for the final projection
    ot = [pxt.tile([128, NQ], MMDT, tag="ot", bufs=8, name=f"ot{i}")
          for i in range(NT_E)]
    # q slice s covers q [s*256, s*256+256) = kv rows [s*256+128, s*256+384);
    # its window reaches kv [s*256, s*256+512) = kv tiles [2s, 2s+4) exactly.
    for h in range(NHEAD):
        th, off = h // 2, (h % 2) * DHEAD
        for s in range(NSL):
            qsl = ts(s, QSL)
            sets = []
            for j in range(4):
                kvt = 2 * s + j
                pe = pps.tile([128, QSL], F32, tag="ps")
                nc.tensor.matmul(pe[:],
                                 r(kt[th][off:off + DHEAD, ts(kvt, 128)]),
                                 r(qt[th][off:off + DHEAD, qsl]),
                                 start=True, stop=True)
                st = pset.tile([128, QSL], MMDT, tag="set")
                nc.scalar.activation(st[:], pe[:], AF.Exp, scale=float(SCALE))
                nc.vector.tensor_mul(st[:], st[:], maskt[kvt][:, qsl])
                sets.append(st)
            po = ppv.tile([DHEAD + 1, QSL], F32, tag="pv")
            for j in range(4):
                nc.tensor.matmul(po[:],
                                 r(v[2 * s + j][:, h * 65:h * 65 + 65]),
                                 r(sets[j][:]),
                                 start=(j == 0), stop=(j == 3))
            rc = psml.tile([1, QSL], F32, tag="recip", bufs=3,
                           name=f"rc{h}_{s}")
            nc.vector.reciprocal(rc[:], po[DHEAD:DHEAD + 1, :])
            rb = prcb.tile([DHEAD, QSL], F32, tag="rcb", name=f"rb{h}_{s}")
            nc.sync.dma_start(
                rb[:], rc[:].unsqueeze(1).broadcast_to((1, DHEAD, QSL)))
            nc.vector.tensor_mul(ot[th][off:off + DHEAD, qsl],
                                 po[0:DHEAD, :], rb[:])
            nc.vector.tensor_scalar_add(ot[th][off:off + DHEAD, qsl],
                                        ot[th][off:off + DHEAD, qsl],
                                        bvs[off:off + DHEAD, th:th + 1])

    # ---- final projection: out = O @ wo + bo ----
    for q_i in range(NT_Q):
        fin = pfin.tile([128, EMB], F32, tag="fin")
        for ch in range(2):
            pf = pps.tile([128, 512], F32, tag="ps")
            for k in range(NT_E):
                nc.tensor.matmul(pf[:], r(ot[k][:, ts(q_i, 128)]),
                                 r(wo_t[k][:, ts(ch, 512)]),
                                 start=(k == 0), stop=(k == NT_E - 1))
            nc.vector.tensor_add(fin[:, ts(ch, 512)], pf[:],
                                 bob[:, ts(ch, 512)])
        nc.sync.dma_start(aps["out"][ts(q_i, 128), :], fin[:])


_NC_CACHE = {}


def _build_nc():
    key = ("nc", USE_F32R)
    if key in _NC_CACHE:
        return _NC_CACHE[key]
    nc = bacc.Bacc("TRN2", target_bir_lowering=False, debug=False,
                   enable_asserts=False, num_devices=8)
    MMDT = F32R if USE_F32R else F32
    aps = {}
    for name, shape, dt_ in [("xtkv", [EMB, NKV], MMDT),
                             ("mask", [NKV, NQ], F32),
                             ("wq", [EMB, EMB], MMDT), ("wk", [EMB, EMB], MMDT),
                             ("wv", [EMB, EMB], MMDT), ("wo", [EMB, EMB], MMDT),
                             ("bqc", [128, NT_E], F32), ("bkc", [128, NT_E], F32),
                             ("bvc", [128, NT_E], F32), ("bob", [128, EMB], F32),
                             ("onesc", [128, NHEAD], MMDT)]:
        aps[name] = nc.dram_tensor(name, shape, dt_, kind="ExternalInput").ap()
    aps["out"] = nc.dram_tensor("out", [NQ, EMB], F32,
                                kind="ExternalOutput").ap()
    with tile.TileContext(nc) as tc:
        with ExitStack() as ctx:
            _body(ctx, tc, aps)
    nc.compile()
    _NC_CACHE[key] = nc
    return nc


def _shard_inputs(x, wq, bq, wk, bk, wv, bv, wo, bo):
    x = np.asarray(x, dtype=np.float32)
    arrs = {n: np.ascontiguousarray(np.asarray(a, dtype=np.float32))
            for n, a in [("wq", wq), ("wk", wk), ("wv", wv), ("wo", wo)]}
    bq, bk, bv, bo = (np.asarray(b, dtype=np.float32) for b in (bq, bk, bv, bo))
    arrs["bqc"] = np.ascontiguousarray(bq.reshape(NT_E, 128).T)
    arrs["bkc"] = np.ascontiguousarray(bk.reshape(NT_E, 128).T)
    arrs["bvc"] = np.ascontiguousarray(bv.reshape(NT_E, 128).T)
    arrs["bob"] = np.ascontiguousarray(np.broadcast_to(bo, (128, EMB)))
    arrs["onesc"] = np.ones((128, NHEAD), dtype=np.float32)
    in_maps = []
    for core in range(8):
        b, c = core // 4, core % 4
        qs = c * NQ
        k0 = qs - QOFF  # first kv row; may be out of range (zero-padded)
        m = dict(arrs)
        xt = np.zeros((NKV, EMB), dtype=np.float32)
        lo, hi = max(0, k0), min(SEQ, k0 + NKV)
        xt[lo - k0:hi - k0, :] = x[b, lo:hi, :]
        m["xtkv"] = np.ascontiguousarray(xt.T)
        kpos = k0 + np.arange(NKV)[:, None]
        qpos = qs + np.arange(NQ)[None, :]
        m["mask"] = ((np.abs(kpos - qpos) <= WIN)
                     & (kpos >= 0) & (kpos < SEQ)).astype(np.float32)
        in_maps.append(m)
    return in_maps


def kernel(x, wq, bq, wk, bk, wv, bv, wo, bo):
    global LAST_RESULT
    nc = _build_nc()
    in_maps = _shard_inputs(x, wq, bq, wk, bk, wv, bv, wo, bo)
    res = run_bass_kernel_spmd(nc, in_maps, core_ids=list(range(8)))
    LAST_RESULT = res
    out = np.empty((BATCH, SEQ, EMB), dtype=np.float32)
    for core in range(8):
        b, c = core // 4, core % 4
        out[b, c * NQ:(c + 1) * NQ, :] = res.results[core]["out"]
    return out



# revision 23
# speedup vs baseline: 1.0935x; 1.0935x over previous
"""LocalAttention Trainium2 Bass kernel (v2, bf16).

Problem: x (2, 2048, 1024) f32 -> QKV proj (16 heads, d=64), local-window
attention (|i-j| <= 128), output projection.

Sharding (no collectives): 8 shards = 2 batches x 4 seq-chunks of 512 q rows.
Each core gets a uniform 768-row KV halo slice [qs-128, qs+640), zero-padded
at the sequence edges, so q row i == kv row i+128 on every core.

v2 changes vs v1 (254.6us):
  * all matmul operands bf16 (full PE rate, FWL weight loads, half DMA)
  * energy tiles j0/j3 merged: for q slice [qs,qs+256) kv tile j0 only
    covers q cols 0:128 and j3 only 128:256, so both live in one PSUM tile
    -> 25% less energy/PV matmul + exp + mask work
  * heads processed in pairs: even head on PE row/col groups 0-1, odd head
    on 64:128 (row-tiled energy, col-tiled PV) so the pair runs concurrent
    on the PE and PV outputs land on the partitions OT needs (no cross-lane
    copies; engines cannot shift partitions)
  * softmax denominators via selector matmuls (lhsT = one-hot column per
    head) accumulating all 16 heads into one [16, 512] PSUM tile, one
    batched reciprocal per slice, broadcast-DMA + single multiply per OT
    tile for normalization (v1 spent 55us in per-head [1,256] reciprocals)
  * V bias folded into the V tiles (softmax rows sum to 1 so
    softmax@(V+bv) = softmax@V + bv), removing the per-head bias adds
  * weight DMAs interleaved with x so the first projection matmul can
    start ~10us in instead of 25us
"""

import os
from contextlib import ExitStack

import numpy as np

import concourse.bacc as bacc
import concourse.mybir as mybir
import concourse.tile as tile
from concourse.bass_utils import run_bass_kernel_spmd

F32 = mybir.dt.float32
BF16 = mybir.dt.bfloat16
AF = mybir.ActivationFunctionType

PHASE = int(os.environ.get("KERNEL_PHASE", "4"))  # bisect aid: 1..4

EMB = 1024
NHEAD = 16
DHEAD = 64
WIN = 128
BATCH = 2
SEQ = 2048
NQ = 512          # q rows per core
NKV = 768         # kv rows per core (q rows + clamped 128 halo each side)
NT_E = EMB // 128   # 8 e-tiles
NT_KV = NKV // 128  # 6 kv-tiles
NT_Q = NQ // 128    # 4 q-tiles
QOFF = 128          # q row i == kv row i + QOFF, uniformly
NSL = 2             # q slices per core (256 wide)
QSL = NQ // NSL     # 256
NPAIR = NHEAD // 2  # 8 head pairs
SCALE = 1.0 / np.sqrt(EMB)       # energy / sqrt(emb)

LAST_RESULT = None  # BassKernelResults of the most recent run (for profiling)


def _body(ctx, tc, aps):
    nc = tc.nc

    pw = ctx.enter_context(tc.tile_pool(name="w", bufs=32))
    pxt = ctx.enter_context(tc.tile_pool(name="xt", bufs=8))
    pqt = ctx.enter_context(tc.tile_pool(name="qt", bufs=8))
    pkt = ctx.enter_context(tc.tile_pool(name="kt", bufs=16))
    pv = ctx.enter_context(tc.tile_pool(name="v", bufs=6))
    pst = ctx.enter_context(tc.tile_pool(name="st", bufs=8))
    pmask = ctx.enter_context(tc.tile_pool(name="mask", bufs=6))
    pot = ctx.enter_context(tc.tile_pool(name="ot", bufs=8))
    psml = ctx.enter_context(tc.tile_pool(name="small", bufs=1))
    pbb = ctx.enter_context(tc.tile_pool(name="bb", bufs=2))
    prb = ctx.enter_context(tc.tile_pool(name="rb", bufs=4))
    pfin = ctx.enter_context(tc.tile_pool(name="fin", bufs=2))
    pps = ctx.enter_context(tc.tile_pool(name="ps", bufs=2, space="PSUM"))
    pse = ctx.enter_context(tc.tile_pool(name="pse", bufs=3, space="PSUM"))
    ppv = ctx.enter_context(tc.tile_pool(name="pspv", bufs=2, space="PSUM"))
    ppd = ctx.enter_context(tc.tile_pool(name="psdn", bufs=1, space="PSUM"))

    ts = lambda i, s: slice(i * s, (i + 1) * s)  # noqa: E731

    # ---- small constants ----
    bqs = psml.tile([128, NT_E], F32, tag="bias", bufs=3)
    nc.sync.dma_start(bqs[:], aps["bqc"][:])
    bks = psml.tile([128, NT_E], F32, tag="bias", bufs=3)
    nc.sync.dma_start(bks[:], aps["bkc"][:])
    bvb = pbb.tile([128, EMB], BF16, tag="bvb")
    nc.sync.dma_start(bvb[:], aps["bvb"][:])
    bob = pbb.tile([128, EMB], F32, tag="bob")
    nc.sync.dma_start(bob[:], aps["bob"][:])
    sel = psml.tile([128, NHEAD * NHEAD], BF16, tag="sel", bufs=1)
    nc.sync.dma_start(sel[:], aps["sel"][:])

    # ---- x + wq interleaved so Q proj can start early ----
    def load_w(name):
        tiles = []
        for t in range(NT_E):
            w = pw.tile([128, EMB], BF16, tag="w")
            nc.sync.dma_start(w[:], aps[name][ts(t, 128), :])
            tiles.append(w)
        return tiles

    wq_t, xtkv = [], []
    for t in range(NT_E):
        w = pw.tile([128, EMB], BF16, tag="w")
        nc.sync.dma_start(w[:], aps["wq"][ts(t, 128), :])
        wq_t.append(w)
        xk = pxt.tile([128, NKV], BF16, tag="xtkv")
        nc.sync.dma_start(xk[:], aps["xtkv"][ts(t, 128), :])
        xtkv.append(xk)

    # ---- QT = (x_q @ wq + bq)^T : [dim, q] bf16 ----
    qt = []
    for t in range(NT_E):
        ps = pps.tile([128, NQ], F32, tag="ps")
        for k in range(NT_E):
            nc.tensor.matmul(ps[:], wq_t[k][:, ts(t, 128)],
                             xtkv[k][:, QOFF:QOFF + NQ],
                             start=(k == 0), stop=(k == NT_E - 1))
        q = pqt.tile([128, NQ], BF16, tag="qt")
        nc.scalar.activation(q[:], ps[:], AF.Identity, bias=bqs[:, t:t + 1])
        qt.append(q)

    # ---- KT = (x_kv @ wk + bk)^T, zero-padded per head ----
    # kth[2t] holds head 2t's dims on partitions 0:64 and ZEROS on 64:128
    # (kth[2t+1] the reverse), so energy matmuls contract over the full
    # K=128 with the wrong head killed by zeros. K=64 bf16 matmuls (the
    # direct approach) crash this HW path, K=128 is proven.
    wk_t = load_w("wk")
    kth = [pkt.tile([128, NKV], BF16, tag="kt", name=f"kth{h}")
           for h in range(NHEAD)]
    for h in range(NHEAD):
        z = slice(64, 128) if h % 2 == 0 else slice(0, 64)
        nc.vector.memset(kth[h][z, :], 0.0)
    for t in range(NT_E):
        for ch, (c0, cw) in enumerate([(0, 512), (512, 256)]):
            ps = pps.tile([128, cw], F32, tag="ps")
            for k in range(NT_E):
                nc.tensor.matmul(ps[:], wk_t[k][:, ts(t, 128)],
                                 xtkv[k][:, c0:c0 + cw],
                                 start=(k == 0), stop=(k == NT_E - 1))
            nc.scalar.activation(kth[2 * t][0:64, c0:c0 + cw], ps[0:64, :],
                                 AF.Identity, bias=bks[0:64, t:t + 1])
            nc.scalar.activation(kth[2 * t + 1][64:128, c0:c0 + cw],
                                 ps[64:128, :],
                                 AF.Identity, bias=bks[64:128, t:t + 1])

    # ---- V = x_kv @ wv + bv : [kv, dim] bf16 (bias folded in;
    #      softmax rows sum to 1 so softmax@(V+bv) = out + bv) ----
    wv_t = load_w("wv")
    v = []
    for kvt in range(NT_KV):
        vt = pv.tile([128, EMB], BF16, tag="v")
        for half in range(2):
            ps = pps.tile([128, 512], F32, tag="ps")
            for k in range(NT_E):
                nc.tensor.matmul(ps[:], xtkv[k][:, ts(kvt, 128)],
                                 wv_t[k][:, ts(half, 512)],
                                 start=(k == 0), stop=(k == NT_E - 1))
            nc.vector.tensor_add(vt[:, ts(half, 512)], ps[:],
                                 bvb[:, ts(half, 512)])
        v.append(vt)

    # ---- masks + wo (needed later; load behind the weights above) ----
    maskt = {}
    for s in range(NSL):
        for jj in range(3):
            m = pmask.tile([128, 512], BF16, tag="mask")
            nc.sync.dma_start(m[:], aps["msk"][ts(s * 3 + jj, 128), :])
            maskt[(s, jj)] = m
    wo_t = load_w("wo")

    # ---- attention: head pairs, merged j0/j3 energy tiles ----
    # q slice s covers q [256s, 256s+256) = kv rows [256s+128, 256s+384);
    # window reaches kv tiles 2s..2s+3. Tile 2s (j0) only matters for
    # q cols 0:128 of the slice, tile 2s+3 (j3) only for 128:256.
    ot = [pot.tile([128, NQ], BF16, tag="ot", name=f"ot{i}")
          for i in range(NPAIR)]
    pd = ppd.tile([NHEAD, NQ], F32, tag="pd")  # denominators, all heads

    if PHASE < 2:  # bisect: stop after projections
        for q_i in range(NT_Q):
            fin = pfin.tile([128, EMB], F32, tag="fin")
            nc.vector.memset(fin[:], 0.0)
            nc.sync.dma_start(aps["out"][ts(q_i, 128), :], fin[:])
        return

    for s in range(NSL):
        qsl = ts(s, QSL)
        qh = [slice(s * QSL, s * QSL + 128), slice(s * QSL + 128, (s + 1) * QSL)]
        tj = [2 * s + j for j in range(4)]
        for p in range(NPAIR):
            offs = [0, DHEAD]  # head 2p on partitions 0:64, head 2p+1 on 64:128
            # energy: pe03 = [h0(j0 q0:128 | j3 q128:256) | h1(same)],
            # pe1/pe2 = [h0 q0:256 | h1 q0:256]. Full K=128 contraction with
            # the zero-padded per-head KT tiles; qt holds both heads' rows.
            pe03 = pse.tile([128, 512], F32, tag="pse")
            pe1 = pse.tile([128, 512], F32, tag="pse")
            pe2 = pse.tile([128, 512], F32, tag="pse")
            for j, pe, cols in ((1, pe1, [slice(0, 256), slice(256, 512)]),
                                (2, pe2, [slice(0, 256), slice(256, 512)]),
                                (0, pe03, [slice(0, 128), slice(256, 384)]),
                                (3, pe03, [slice(128, 256), slice(384, 512)])):
                for h in range(2):
                    rhs_q = qsl if j in (1, 2) else qh[0 if j == 0 else 1]
                    nc.tensor.matmul(pe[:, cols[h]],
                                     kth[2 * p + h][:, ts(tj[j], 128)],
                                     qt[p][:, rhs_q],
                                     start=True, stop=True)
            # exp + mask (st is bf16)
            sts = []
            for jj, pe in ((0, pe03), (1, pe1), (2, pe2)):
                st = pst.tile([128, 512], BF16, tag="st")
                if PHASE == 21:  # bisect: consume pe with a plain copy
                    nc.vector.tensor_copy(st[:], pe[:])
                else:
                    nc.scalar.activation(st[:], pe[:], AF.Exp, scale=float(SCALE))
                if PHASE not in (21, 22):
                    nc.vector.tensor_mul(st[:], st[:], maskt[(s, jj)][:])
                sts.append(st)
            st03, st1, st2 = sts
            if PHASE < 3 or PHASE in (21, 22):  # bisect: stop before PV
                continue
            # PV into one PSUM bank: h0 dims -> [0:64, 0:256] via PE col
            # group 0/1, h1 dims -> [64:128, 256:512] via col group 2/3
            # (so the pair runs concurrent and lands where OT needs it)
            po = ppv.tile([128, 512], F32, tag="pspv")
            hc = [slice(0, 256), slice(256, 512)]
            for h in range(2):
                vc = lambda t_: v[t_][:, (2 * p + h) * DHEAD:
                                      (2 * p + h + 1) * DHEAD]  # noqa: E731
                od = slice(offs[h], offs[h] + DHEAD)
                # j1 first (covers the full 256 q cols), then the partial
                # tiles accumulate inside the already-written region.
                nc.tensor.matmul(po[od, hc[h]], vc(tj[1]), st1[:, hc[h]],
                                 start=True, stop=False)
                nc.tensor.matmul(po[od, hc[h].start:hc[h].start + 128],
                                 vc(tj[0]), st03[:, hc[h].start:hc[h].start + 128],
                                 start=False, stop=False)
                nc.tensor.matmul(po[od, hc[h]], vc(tj[2]), st2[:, hc[h]],
                                 start=False, stop=False)
                nc.tensor.matmul(po[od, hc[h].start + 128:hc[h].stop],
                                 vc(tj[3]), st03[:, hc[h].start + 128:hc[h].stop],
                                 start=False, stop=True)
                # denominator: selector lhsT puts sum(st) in row 2p+h of pd
                # and exact zeros in every other row, so all 16 heads
                # accumulate into the same [16, 256] region.
                r = 2 * p + h
                sl = sel[:, r * NHEAD:(r + 1) * NHEAD]
                q0 = s * QSL
                first = (p == 0 and h == 0)
                last = (p == NPAIR - 1 and h == 1)
                nc.tensor.matmul(pd[:, q0:q0 + QSL], sl, st1[:, hc[h]],
                                 start=first, stop=False)
                nc.tensor.matmul(pd[:, q0:q0 + 128], sl,
                                 st03[:, hc[h].start:hc[h].start + 128],
                                 start=False, stop=False)
                nc.tensor.matmul(pd[:, q0:q0 + QSL], sl, st2[:, hc[h]],
                                 start=False, stop=False)
                nc.tensor.matmul(pd[:, q0 + 128:q0 + QSL], sl,
                                 st03[:, hc[h].start + 128:hc[h].stop],
                                 start=False, stop=last)
            # evict unnormalized head outputs (no partition shift needed)
            nc.vector.tensor_copy(ot[p][0:DHEAD, qsl], po[0:DHEAD, 0:256])
            nc.vector.tensor_copy(ot[p][DHEAD:128, qsl], po[DHEAD:128, 256:512])

    if PHASE < 4 or PHASE in (21, 22):  # bisect: skip norm + out proj
        for q_i in range(NT_Q):
            fin = pfin.tile([128, EMB], F32, tag="fin")
            nc.vector.memset(fin[:], 0.0)
            nc.sync.dma_start(aps["out"][ts(q_i, 128), :], fin[:])
        return

    # ---- normalization + output projection, per slice ----
    for s in range(NSL):
        qsl = ts(s, QSL)
        rc = psml.tile([NHEAD, QSL], F32, tag="rc", bufs=2, name=f"rc{s}")
        nc.vector.reciprocal(rc[:], pd[:, qsl])
        for p in range(NPAIR):
            rb = prb.tile([128, QSL], F32, tag="rb")
            nc.sync.dma_start(
                rb[:],
                rc[2 * p:2 * p + 2, :].unsqueeze(1).broadcast_to((2, DHEAD, QSL)))
            nc.vector.tensor_mul(ot[p][:, qsl], ot[p][:, qsl], rb[:])
        for q_i in (2 * s, 2 * s + 1):
            fin = pfin.tile([128, EMB], F32, tag="fin")
            for ch in range(2):
                pf = pps.tile([128, 512], F32, tag="ps")
                for k in range(NT_E):
                    nc.tensor.matmul(pf[:], ot[k][:, ts(q_i, 128)],
                                     wo_t[k][:, ts(ch, 512)],
                                     start=(k == 0), stop=(k == NT_E - 1))
                nc.vector.tensor_add(fin[:, ts(ch, 512)], pf[:],
                                     bob[:, ts(ch, 512)])
            nc.sync.dma_start(aps["out"][ts(q_i, 128), :], fin[:])


_NC_CACHE = {}


def _build_nc():
    key = ("nc_v2", PHASE)
    if key in _NC_CACHE:
        return _NC_CACHE[key]
    nc = bacc.Bacc("TRN2", target_bir_lowering=False, debug=False,
                   enable_asserts=False, num_devices=8)
    aps = {}
    for name, shape, dt_ in [("xtkv", [EMB, NKV], BF16),
                             ("msk", [NSL * 3 * 128, 512], BF16),
                             ("wq", [EMB, EMB], BF16), ("wk", [EMB, EMB], BF16),
                             ("wv", [EMB, EMB], BF16), ("wo", [EMB, EMB], BF16),
                             ("bqc", [128, NT_E], F32), ("bkc", [128, NT_E], F32),
                             ("bvb", [128, EMB], BF16), ("bob", [128, EMB], F32),
                             ("sel", [128, NHEAD * NHEAD], BF16)]:
        aps[name] = nc.dram_tensor(name, shape, dt_, kind="ExternalInput").ap()
    aps["out"] = nc.dram_tensor("out", [NQ, EMB], F32,
                                kind="ExternalOutput").ap()
    with tile.TileContext(nc) as tc:
        with ExitStack() as ctx:
            _body(ctx, tc, aps)
    nc.compile()
    _NC_CACHE[key] = nc
    return nc


def _shard_inputs(x, wq, bq, wk, bk, wv, bv, wo, bo):
    BF = mybir.dt.np(BF16)
    x = np.asarray(x, dtype=np.float32).astype(BF)
    arrs = {n: np.ascontiguousarray(np.asarray(a, dtype=np.float32).astype(BF))
            for n, a in [("wq", wq), ("wk", wk), ("wv", wv), ("wo", wo)]}
    bq, bk, bv, bo = (np.asarray(b, dtype=np.float32) for b in (bq, bk, bv, bo))
    arrs["bqc"] = np.ascontiguousarray(bq.reshape(NT_E, 128).T)
    arrs["bkc"] = np.ascontiguousarray(bk.reshape(NT_E, 128).T)
    arrs["bvb"] = np.ascontiguousarray(
        np.broadcast_to(bv, (128, EMB)).astype(BF))
    arrs["bob"] = np.ascontiguousarray(np.broadcast_to(bo, (128, EMB)))
    sel = np.zeros((128, NHEAD * NHEAD), dtype=BF)
    for r in range(NHEAD):
        sel[:, r * NHEAD + r] = 1.0
    arrs["sel"] = sel
    in_maps = []
    for core in range(8):
        b, c = core // 4, core % 4
        qs = c * NQ
        k0 = qs - QOFF  # first kv row; may be out of range (zero-padded)
        m = dict(arrs)
        xt = np.zeros((NKV, EMB), dtype=BF)
        lo, hi = max(0, k0), min(SEQ, k0 + NKV)
        xt[lo - k0:hi - k0, :] = x[b, lo:hi, :]
        m["xtkv"] = np.ascontiguousarray(xt.T)
        kpos = k0 + np.arange(NKV)[:, None]
        qpos = qs + np.arange(NQ)[None, :]
        base = ((np.abs(kpos - qpos) <= WIN)
                & (kpos >= 0) & (kpos < SEQ)).astype(BF)
        msk = np.zeros((NSL * 3, 128, 512), dtype=BF)
        for s in range(NSL):
            q0 = s * QSL
            m1 = base[128 * (2 * s + 1):128 * (2 * s + 2), q0:q0 + QSL]
            m2 = base[128 * (2 * s + 2):128 * (2 * s + 3), q0:q0 + QSL]
            m03 = np.concatenate(
                [base[128 * (2 * s):128 * (2 * s + 1), q0:q0 + 128],
                 base[128 * (2 * s + 3):128 * (2 * s + 4), q0 + 128:q0 + QSL]],
                axis=1)
            msk[s * 3 + 0] = np.concatenate([m03, m03], axis=1)
            msk[s * 3 + 1] = np.concatenate([m1, m1], axis=1)
            msk[s * 3 + 2] = np.concatenate([m2, m2], axis=1)
        m["msk"] = np.ascontiguousarray(msk.reshape(NSL * 3 * 128, 512))
        in_maps.append(m)
    return in_maps


def kernel(x, wq, bq, wk, bk, wv, bv, wo, bo):
    global LAST_RESULT
    nc = _build_nc()
    in_maps = _shard_inputs(x, wq, bq, wk, bk, wv, bv, wo, bo)
    res = run_bass_kernel_spmd(nc, in_maps, core_ids=list(range(8)))
    LAST_RESULT = res
    out = np.empty((BATCH, SEQ, EMB), dtype=np.float32)
    for core in range(8):
        b, c = core // 4, core % 4
        out[b, c * NQ:(c + 1) * NQ, :] = res.results[core]["out"]
    return out
